# revision 16
# baseline (speedup 1.0000x reference)
"""Trainium2 Bass kernel for nn_EulerWithEchoModel (B=16,S=2048,V=4096,D=32,H=4).

Math: the reference's per-step update
    th_r = h_r/wav + b + tp ; th_i = h_i/wav + b + tp
    h_r' = cos(th_r)cos(th_i) - sin(th_r)sin(th_i) = cos(th_r + th_i)
    h_i' = sin(th_r + th_i)
collapses (for exact trig; the reference's 4096-entry lerp LUT is within
3e-7 of exact) to a scalar recurrence per (b, d) lane over theta_t =
th_r + th_i.  Working in turns with v = sin(2*pi*tau):
    tau_t = A_t * v_{t-1} + C_t   (mod 1)
    A_t = (sqrt(2)/2pi) / (1 + |w_t|)
    C_t = (2 b_t + 2 t phi + pi/4) / 2pi    (pre-wrapped)
    h_r,t = sin(2pi tau_t + pi/4 - pi/4*2)... recovered as
    h_r = sin(2pi(tau + 1/8)), h_i = sin(2pi(tau - 1/8))
The map is contracting on average (Lyapunov ~ -0.43/step), so time-chunks
of length L run in parallel, each warmed up from zero state for W steps
(validated vs the fp32 jax reference: W=128 reaches the sequential noise
floor).

Echo memory m is a first-order linear recurrence -> tensor_tensor_scan.
Output projection: fp32r matmuls (r/i halves accumulate in PSUM).

Sharding: data-parallel over batch, 2 batch rows per core on 8 cores.
"""
import numpy as np

B, S, V, D, H = 16, 2048, 4096, 32, 4
DH = D // H
D2 = 2 * D
PHI = (1.0 + 5.0 ** 0.5) / 2.0
TWO_PI = 2.0 * float(np.pi)
MAGIC = 1.5 * 2.0 ** 23          # fp32 round-to-nearest-integer magic
N_CORES = 8
BL = B // N_CORES                # batch rows per core (2)
NTOK = BL * S                    # tokens per core (4096)

L = 64                           # scan chunk length
W = 128                          # warmup steps
NCH = S // L                     # chunks per batch row (32)
PADW = W + S                     # padded per-row time length (2176)
N_CHAINS = 2                     # interleaved scan chains (hide ACT latency)
CPC = NCH // N_CHAINS            # chunks per chain per batch row
CHW = S // N_CHAINS              # tokens per chain per batch row (1024)

VT = 512                         # v-tile width (PSUM bank)
TCH = 128                        # token-chunk (PSUM partitions)

_CACHE = {}
LAST_RESULTS = None


def _tpc_const():
    t = np.arange(S, dtype=np.float64)
    x = (2.0 * t * PHI + np.pi / 4.0) / (2.0 * np.pi)
    x = x - np.round(x)
    return x.astype(np.float32)


def _wrap_ids(ids_flat):
    # ap_gather index layout: index #t -> partition t%16, col t//16,
    # replicated for each 16-partition group of the 64 channels
    wrap = np.zeros((16, NTOK // 16), np.int16)
    wrap[np.arange(NTOK) % 16, np.arange(NTOK) // 16] = ids_flat.astype(np.int16)
    return np.tile(wrap, (D // 16, 1))


def _build(with_bias):
    from contextlib import ExitStack
    import concourse.tile as tile
    from concourse import bacc, mybir

    f32 = mybir.dt.float32
    f32r = mybir.dt.float32r
    i16 = mybir.dt.int16
    Sin = mybir.ActivationFunctionType.Sin
    Tanh = mybir.ActivationFunctionType.Tanh
    Abs = mybir.ActivationFunctionType.Abs
    Ident = mybir.ActivationFunctionType.Identity
    ADD = mybir.AluOpType.add
    SUB = mybir.AluOpType.subtract
    MUL = mybir.AluOpType.mult

    nc = bacc.Bacc("TRN2", target_bir_lowering=False, debug=False)

    def register_const(value, dtype=f32):
        key = (dtype, float(value))
        if key in nc.const_aps.aps:
            return
        t = nc.alloc_sbuf_tensor(f"constk-{value}", [128, 1], dtype)
        nc.gpsimd.memset(t.ap(), value)
        nc.const_aps.aps[key] = t.ap()

    register_const(MAGIC)
    register_const(0.125)
    register_const(-0.125)
    nc.all_engine_barrier()

    # ---------------- DRAM I/O ----------------
    embwT_d = nc.dram_tensor("embwT", [D, V], f32, kind="ExternalInput").ap()
    embbT_d = nc.dram_tensor("embbT", [D, V], f32, kind="ExternalInput").ap()
    idsw_d = nc.dram_tensor("idsw", [D, NTOK // 16], i16, kind="ExternalInput").ap()
    tpc_d = nc.dram_tensor("tpc", [D, NTOK], f32, kind="ExternalInput").ap()
    WrT_d = nc.dram_tensor("WrT", [D, V], f32, kind="ExternalInput").ap()
    WiT_d = nc.dram_tensor("WiT", [D, V], f32, kind="ExternalInput").ap()
    wgr_d = nc.dram_tensor("wgr", [D, H], f32, kind="ExternalInput").ap()
    wgi_d = nc.dram_tensor("wgi", [D, H], f32, kind="ExternalInput").ap()
    bg_d = nc.dram_tensor("bg", [H, 1], f32, kind="ExternalInput").ap()
    dec_d = nc.dram_tensor("dec", [D, 1], f32, kind="ExternalInput").ap()
    es_d = nc.dram_tensor("es", [D, 1], f32, kind="ExternalInput").ap()
    eg_d = nc.dram_tensor("eg", [H, D], f32, kind="ExternalInput").ap()
    if with_bias:
        bo_d = nc.dram_tensor("bo", [1, V], f32, kind="ExternalInput").ap()
    out_d = nc.dram_tensor("out", [BL, S, V], f32, kind="ExternalOutput").ap()

    with tile.TileContext(nc) as tc, ExitStack() as ctx:
        cst = ctx.enter_context(tc.tile_pool(name="cst", bufs=1))
        big = ctx.enter_context(tc.tile_pool(name="big", bufs=1))
        sml = ctx.enter_context(tc.tile_pool(name="sml", bufs=1))
        ps_mm = ctx.enter_context(tc.tile_pool(name="psmm", bufs=4, space="PSUM"))
        ps_sm = ctx.enter_context(tc.tile_pool(name="pssm", bufs=2, space="PSUM"))

        # persistent across phases
        WrTr = cst.tile([D, V], f32r)
        WiTr = cst.tile([D, V], f32r)
        if with_bias:
            bor = cst.tile([1, V], f32r)
            ones1r = cst.tile([1, TCH], f32r)
        Apad = big.tile([D, BL * PADW], f32)
        Cpad = big.tile([D, BL * PADW], f32)
        pbuf = [big.tile([D, BL * CHW], f32, tag=f"pbuf{h}", name=f"pbuf{h}")
                for h in range(N_CHAINS)]

        wgr = cst.tile([D, H], f32)
        nc.sync.dma_start(wgr[:], wgr_d[:])
        wgi = cst.tile([D, H], f32)
        nc.sync.dma_start(wgi[:], wgi_d[:])
        bg = cst.tile([H, 1], f32)
        nc.sync.dma_start(bg[:], bg_d[:])
        dec = cst.tile([D, 1], f32)
        nc.sync.dma_start(dec[:], dec_d[:])
        es = cst.tile([D, 1], f32)
        nc.sync.dma_start(es[:], es_d[:])
        eg = cst.tile([H, D], f32)
        nc.sync.dma_start(eg[:], eg_d[:])

        # gate bias halved; decay lambda = sigmoid(dec) via tanh
        bgh = cst.tile([H, 1], f32)
        nc.vector.tensor_scalar(bgh[:], bg[:], 0.5, None, MUL)
        lam = cst.tile([D, 1], f32)
        nc.scalar.activation(lam[:], dec[:], Tanh, scale=0.5)
        nc.vector.tensor_scalar(lam[:], lam[:], 0.5, 0.5, MUL, ADD)

        # ------- prelude (scoped pool: space reclaimed before epilogue) --
        with tc.tile_pool(name="pre", bufs=1) as pre:
            embwT = pre.tile([D, V], f32, tag="sA")
            nc.sync.dma_start(embwT[:], embwT_d[:])
            embbT = pre.tile([D, V], f32, tag="sB")
            nc.sync.dma_start(embbT[:], embbT_d[:])
            idsw = pre.tile([D, NTOK // 16], i16)
            nc.sync.dma_start(idsw[:], idsw_d[:])
            tpc = pre.tile([D, NTOK], f32)
            nc.sync.dma_start(tpc[:], tpc_d[:])
            WrT = pre.tile([D, V], f32, tag="sW")
            nc.sync.dma_start(WrT[:], WrT_d[:])
            nc.scalar.copy(WrTr[:], WrT[:])
            WiT = pre.tile([D, V], f32, tag="sW")
            nc.sync.dma_start(WiT[:], WiT_d[:])
            nc.scalar.copy(WiTr[:], WiT[:])
            if with_bias:
                bo = pre.tile([1, V], f32)
                nc.sync.dma_start(bo[:], bo_d[:])
                nc.scalar.copy(bor[:], bo[:])
                ones1 = pre.tile([1, TCH], f32)
                nc.vector.memset(ones1[:], 1.0)
                nc.vector.tensor_copy(ones1r[:], ones1[:])

            # gathers (both halves at base partition 0)
            Gw = pre.tile([D, NTOK], f32)
            nc.gpsimd.ap_gather(
                Gw[:].unsqueeze(2), embwT[:].unsqueeze(2), idsw[:],
                channels=D, num_elems=V, d=1, num_idxs=NTOK,
            )
            Gb = pre.tile([D, NTOK], f32)
            nc.gpsimd.ap_gather(
                Gb[:].unsqueeze(2), embbT[:].unsqueeze(2), idsw[:],
                channels=D, num_elems=V, d=1, num_idxs=NTOK,
            )

            # A-hat / C-hat (zero-padded warmup region)
            nc.vector.memset(Apad[:], 0.0)
            nc.vector.memset(Cpad[:], 0.0)

            wav = pre.tile([D, NTOK], f32, tag="sA")
            nc.scalar.activation(wav[:], Gw[:], Abs)
            nc.vector.tensor_scalar(wav[:], wav[:], 1.0, None, ADD)
            rec = pre.tile([D, NTOK], f32, tag="sB")
            nc.vector.reciprocal(rec[:], wav[:])

            SC = float(np.sqrt(2.0) / TWO_PI)
            for b in range(BL):
                dst_a = Apad[:, b * PADW + W: b * PADW + W + S]
                nc.vector.tensor_scalar(
                    dst_a, rec[:][:, b * S:(b + 1) * S], SC, None, MUL)
                dst_c = Cpad[:, b * PADW + W: b * PADW + W + S]
                nc.vector.scalar_tensor_tensor(
                    dst_c, Gb[:][:, b * S:(b + 1) * S], float(1.0 / np.pi),
                    tpc[:, b * S:(b + 1) * S], MUL, ADD)

        # ---------------- the scan ----------------
        # padded views [D, BL, PADW/L, L]; chain h owns chunks [h*CPC,(h+1)*CPC)
        ap4 = Apad[:].rearrange("p (b k l) -> p b k l", b=BL, l=L)
        cp4 = Cpad[:].rearrange("p (b k l) -> p b k l", b=BL, l=L)

        # per-chain pbuf: chain h holds tokens [h*CHW,(h+1)*CHW) of each row
        pb4 = [pbuf[h][:].rearrange("p (b k l) -> p b k l", b=BL, l=L)
               for h in range(N_CHAINS)]

        vst, scr, qt, t3, kt = ({} for _ in range(5))
        for h in range(N_CHAINS):
            vst[h] = sml.tile([D, BL * CPC], f32, tag=f"vst{h}", name=f"vst{h}")
            nc.vector.memset(vst[h][:], 0.0)
            scr[h] = sml.tile([D, BL * CPC], f32, tag=f"scr{h}", name=f"scr{h}")
            qt[h] = sml.tile([D, BL * CPC], f32, tag=f"qt{h}", name=f"qt{h}")
            t3[h] = sml.tile([D, BL * CPC], f32, tag=f"t3{h}", name=f"t3{h}")
            kt[h] = sml.tile([D, BL * CPC], f32, tag=f"kt{h}", name=f"kt{h}")

        def r3(t):
            return t[:].rearrange("p (b c) -> p b c", b=BL)

        for j in range(W + L):
            blk, off = j // L, j % L
            for h in range(N_CHAINS):
                c0 = h * CPC
                a_sl = ap4[:, :, c0 + blk: c0 + blk + CPC, off]
                c_sl = cp4[:, :, c0 + blk: c0 + blk + CPC, off]
                v3, q3, t33, k3 = r3(vst[h]), r3(qt[h]), r3(t3[h]), r3(kt[h])
                nc.vector.tensor_tensor(q3, v3, a_sl, MUL)
                nc.vector.tensor_tensor(t33, q3, c_sl, ADD)
                nc.vector.tensor_scalar(k3, t33, MAGIC, MAGIC, ADD, SUB)
                if j >= W:
                    dst = pb4[h][:, :, 0:CPC, j - W]
                else:
                    dst = r3(scr[h])
                nc.vector.tensor_tensor(dst, t33, k3, SUB)
                nc.scalar.activation(vst[h][:], dst, Sin, scale=TWO_PI)

        # ---------------- epilogue: per 512-token slab ----------------
        # token layout is b-major; slab s covers tokens [s*VT,(s+1)*VT),
        # all within batch row b = s*VT//S and chain (t0//CHW)
        hsl = ctx.enter_context(tc.tile_pool(name="hsl", bufs=2))
        stg = ctx.enter_context(tc.tile_pool(name="stg", bufs=2))
        NSLAB = NTOK // VT
        m_prev = {"r": None, "i": None}
        for s in range(NSLAB):
            g0 = s * VT
            b = g0 // S
            t0 = g0 % S
            h = t0 // CHW
            frac = pbuf[h][:, b * CHW + (t0 % CHW): b * CHW + (t0 % CHW) + VT]

            # h_r = sin(2pi(frac+1/8)), h_i = sin(2pi(frac-1/8)), wrapped
            hh = {}
            for nm, off in (("r", 0.125), ("i", -0.125)):
                x = hsl.tile([D, VT], f32, tag=f"x{nm}")
                nc.scalar.activation(x[:], frac, Ident, bias=off)
                T = hsl.tile([D, VT], f32, tag=f"T{nm}")
                nc.scalar.activation(T[:], x[:], Ident, bias=MAGIC)
                g = hsl.tile([D, VT], f32, tag=f"g{nm}")
                nc.vector.scalar_tensor_tensor(g[:], T[:], -MAGIC, x[:], ADD, SUB)
                ht = hsl.tile([D, VT], f32, tag=f"h{nm}")
                nc.scalar.activation(ht[:], g[:], Sin, scale=-TWO_PI)
                hh[nm] = ht

            # gate = sigmoid(h_r@Wg_r + h_i@Wg_i + bg) via tanh
            gl = ps_sm.tile([H, VT], f32, space="PSUM", tag="gl")
            nc.tensor.matmul(gl[:], lhsT=wgr[:], rhs=hh["r"][:],
                             start=True, stop=False)
            nc.tensor.matmul(gl[:], lhsT=wgi[:], rhs=hh["i"][:],
                             start=False, stop=True)
            tg = hsl.tile([H, VT], f32, tag="tg")
            nc.scalar.activation(tg[:], gl[:], Tanh, bias=bgh[:, 0:1], scale=0.5)
            gate = hsl.tile([H, VT], f32, tag="gate")
            nc.vector.tensor_scalar(gate[:], tg[:], 0.5, 0.5, MUL, ADD)

            # expand gate heads 4 -> 32 lanes
            ge = ps_sm.tile([D, VT], f32, space="PSUM", tag="ge")
            nc.tensor.matmul(ge[:], lhsT=eg[:], rhs=gate[:], start=True, stop=True)

            # m scans (echo memory), chained across slabs within a batch row
            comb = {}
            for nm in ("r", "i"):
                y = hsl.tile([D, VT], f32, tag=f"y{nm}")
                nc.vector.tensor_tensor(y[:], ge[:], hh[nm][:], MUL)
                m = hsl.tile([D, VT], f32, tag=f"m{nm}")
                init = 0.0 if t0 == 0 else m_prev[nm][:, VT - 1:VT]
                nc.vector.tensor_tensor_scan(
                    m[:], lam[:, 0:1].to_broadcast([D, VT]), y[:], init, MUL, ADD)
                m_prev[nm] = m
                cb = hsl.tile([D, VT], f32r, tag=f"cb{nm}")
                nc.vector.scalar_tensor_tensor(
                    cb[:], m[:], es[:, 0:1], hh[nm][:], MUL, ADD)
                comb[nm] = cb

            # output projection for this slab
            for mc in range(VT // TCH):
                tt0 = t0 + mc * TCH
                stage = stg.tile([TCH, V], f32, tag="stage")
                for vt in range(V // VT):
                    acc = ps_mm.tile([TCH, VT], f32, space="PSUM", tag="acc")
                    nc.tensor.matmul(
                        acc[:], lhsT=comb["r"][:, mc * TCH:(mc + 1) * TCH],
                        rhs=WrTr[:, vt * VT:(vt + 1) * VT],
                        start=True, stop=False)
                    nc.tensor.matmul(
                        acc[:], lhsT=comb["i"][:, mc * TCH:(mc + 1) * TCH],
                        rhs=WiTr[:, vt * VT:(vt + 1) * VT],
                        start=False, stop=not with_bias)
                    if with_bias:
                        nc.tensor.matmul(
                            acc[:], lhsT=ones1r[:],
                            rhs=bor[:, vt * VT:(vt + 1) * VT],
                            start=False, stop=True)
                    if vt % 2 == 0:
                        nc.vector.tensor_copy(stage[:, vt * VT:(vt + 1) * VT], acc[:])
                    else:
                        nc.scalar.copy(stage[:, vt * VT:(vt + 1) * VT], acc[:])
                nc.sync.dma_start(out_d[b, tt0:tt0 + TCH, :], stage[:])

    nc.compile()
    return nc


def _marshal(inputs):
    ids = np.ascontiguousarray(np.asarray(inputs["input_ids"])).astype(np.int64)
    emb = np.asarray(inputs["emb_table"], dtype=np.float32)
    W_out = np.asarray(inputs["W_out"], dtype=np.float32)
    b_out = np.asarray(inputs["b_out"], dtype=np.float32)
    W_gate = np.asarray(inputs["W_gate"], dtype=np.float32)
    b_gate = np.asarray(inputs["b_gate"], dtype=np.float32)
    dec = np.asarray(inputs["decay_logit"], dtype=np.float32)
    es = np.asarray(inputs["echo_scale"], dtype=np.float32)

    with_bias = bool(np.any(b_out != 0.0))
    tpc_half = _tpc_const()
    shared = dict(
        embwT=np.ascontiguousarray(emb[:, :D].T),
        embbT=np.ascontiguousarray(emb[:, D:].T),
        tpc=np.ascontiguousarray(
            np.broadcast_to(np.tile(tpc_half, BL)[None, :], (D, NTOK))),
        WrT=np.ascontiguousarray(W_out[:, :D].T),
        WiT=np.ascontiguousarray(W_out[:, D:].T),
        wgr=np.ascontiguousarray(W_gate[:D]),
        wgi=np.ascontiguousarray(W_gate[D:]),
        bg=b_gate.reshape(H, 1),
        dec=np.repeat(dec, DH).reshape(D, 1),
        es=np.broadcast_to(es.reshape(1, 1), (D, 1)).copy(),
        eg=np.ascontiguousarray(
            (np.arange(D)[None, :] // DH == np.arange(H)[:, None])
            .astype(np.float32)),
    )
    if with_bias:
        shared["bo"] = b_out.reshape(1, V)

    in_maps = []
    for c in range(N_CORES):
        ids_c = ids[c * BL:(c + 1) * BL].reshape(-1)
        m = dict(shared)
        m["idsw"] = _wrap_ids(ids_c)
        in_maps.append(m)
    return in_maps, with_bias


def kernel(**inputs):
    from concourse.bass_utils import run_bass_kernel_spmd

    in_maps, with_bias = _marshal(inputs)
    key = ("nc", with_bias)
    if key not in _CACHE:
        _CACHE[key] = _build(with_bias)
    nc = _CACHE[key]

    res = run_bass_kernel_spmd(nc, in_maps, list(range(N_CORES)))
    out = np.concatenate([r["out"] for r in res.results], axis=0)
    return np.ascontiguousarray(out.reshape(B, S, V))


def bench(inputs, iters=3):
    """Time the on-device execution (dispatch + run, outputs stay on
    device). Mirrors bass2jax.run_bass_via_pjrt's multi-core path."""
    import time
    import jax
    from concourse import bass2jax, mybir
    from jax.experimental.shard_map import shard_map
    from jax.sharding import Mesh, NamedSharding, PartitionSpec

    bass2jax.install_neuronx_cc_hook()
    in_maps, with_bias = _marshal(inputs)
    key = ("nc", with_bias)
    if key not in _CACHE:
        _CACHE[key] = _build(with_bias)
    nc = _CACHE[key]

    part_name = nc.partition_id_tensor.name if nc.partition_id_tensor else None
    in_names, out_names, out_avals, zero_outs = [], [], [], []
    for alloc in nc.m.functions[0].allocations:
        if not isinstance(alloc, mybir.MemoryLocationSet):
            continue
        name = alloc.memorylocations[0].name
        if alloc.kind == "ExternalInput":
            if name != part_name:
                in_names.append(name)
        elif alloc.kind == "ExternalOutput":
            out_names.append(name)
            shape = tuple(alloc.tensor_shape)
            dtype = mybir.dt.np(alloc.dtype)
            out_avals.append(jax.core.ShapedArray(shape, dtype))
            zero_outs.append(np.zeros(shape, dtype))
    n_params = len(in_names)
    n_outs = len(out_avals)
    all_names = in_names + out_names
    if part_name is not None:
        all_names = all_names + [part_name]

    def _body(*args):
        operands = list(args)
        if part_name is not None:
            operands.append(bass2jax.partition_id_tensor())
        outs = bass2jax._bass_exec_p.bind(
            *operands,
            out_avals=tuple(out_avals),
            in_names=tuple(all_names),
            out_names=tuple(out_names),
            lowering_input_output_aliases=(),
            sim_require_finite=True,
            sim_require_nnan=True,
            nc=nc,
        )
        return tuple(outs)

    devices = jax.devices()[:N_CORES]
    mesh = Mesh(np.asarray(devices), ("core",))
    donate = tuple(range(n_params, n_params + n_outs))
    fn = jax.jit(
        shard_map(_body, mesh=mesh,
                  in_specs=(PartitionSpec("core"),) * (n_params + n_outs),
                  out_specs=(PartitionSpec("core"),) * n_outs,
                  check_rep=False),
        donate_argnums=donate, keep_unused=True)

    sh = NamedSharding(mesh, PartitionSpec("core"))
    concat_in = [
        jax.device_put(
            np.concatenate([np.asarray(in_maps[c][nm]) for c in range(N_CORES)],
                           axis=0), sh)
        for nm in in_names
    ]
    jax.block_until_ready(concat_in)

    times = []
    for _ in range(iters):
        zeros_dev = [
            jax.device_put(np.zeros((N_CORES * z.shape[0], *z.shape[1:]),
                                    z.dtype), sh)
            for z in zero_outs
        ]
        jax.block_until_ready(zeros_dev)
        t0 = time.perf_counter()
        out = fn(*concat_in, *zeros_dev)
        jax.block_until_ready(out)
        times.append(time.perf_counter() - t0)
        del out
    return times
